# revision 1
# baseline (speedup 1.0000x reference)
"""Trainium2 Bass kernel for nn_ConvBlock (MuLUT-style conv block), v3.

Math (per reference): see kernel.py baseline docstring. Same sharding: 8 cores
= 8 (rotation, batch) pairs; each core computes all 6 branches over its 4096
rotated+padded pixels; host does im2col prep and the B6/tanh/round/shuffle/
unrotate/sum epilogue.

v3 redesign (vs baseline 10 fp32r streams per branch-chunk):
  - bf16 matmuls (1 cycle/row like fp32r at N=512, but M can exceed 64),
    fp32 psum. Empirical numpy check: bf16-everywhere rel err 5.4e-3 << 2e-2.
  - 8 streams per branch-chunk = ceil(978 useful K rows / 128), the floor:
      s1 conv           [xcol 18+pad]      -> O1[0:64]
      s2 w2             [h0]    K=64       -> A1[0:64]
      s3 w3             [h0;h1] K=128      -> O2[0:64]
      s4 w4 (h01 part)  [h0;h1]            -> A2[0:64]  (start)
      s5 w4 (h2 part)   [h2;h3stale]       -> A2[0:64]  (stop; h3 rows zero-w)
      s6 [w5|w6] h01    [h0;h1]  M=72      -> D[0:72]   (start)
      s7 [w5|w6] h23    [h2;h3]  M=72      -> D[0:72]   (stop)
      s8 w6 (h4 part)   [h4]     M=8       -> D[64:72]  (accum past stop)
    W6 (M=8) rides W5's streams in the M dimension - this is what removes 2
    of the baseline's 10 streams.
  - Chunk-pair merged activations: psum tiles are [128, 1024] (2 banks) so
    each relu is ONE [64,1024] op instead of two [64,512] ops; DVE/ACT are
    free-size-bound, so this halves their per-op overhead count.
  - Per-unit (branch x chunk-pair) ops: 16 matmuls, 5 relus, 1 y-copy.
  - PSUM: tag "x" bufs=3 rotates O1/A1/O2/A2 (handover waits land on already
    drained groups); tag "y" bufs=1 for D. 6+2 = 8 banks exactly.
  - Software-pipelined emission (6 phases, lag=phase, oldest unit first
    within a step) keeps the in-order tensor queue from head-blocking.
"""

import sys

import numpy as np
import ml_dtypes

if "/opt/trn_rl_repo" not in sys.path:
    sys.path.insert(0, "/opt/trn_rl_repo")

IN_C, OUT_C, SCALE, S, NF = 2, 2, 2, 3, 64
MODES = 3
NB = IN_C * MODES
PAD = S - 1
B, H = 2, 64
NPIX = H * H
NCP = 4                 # chunk-pairs per image
CPW = 1024              # pixels per chunk-pair
CW = 512                # pixels per chunk (psum bank limit in fp32)
N_CORES = 8
LANES = 6
WCOLS = 408             # weight cols per branch in the packed wd tensor
BF16 = ml_dtypes.bfloat16

_BASS_CACHE = {}


def _build_bass():
    import concourse.bass as bass  # noqa: F401
    import concourse.mybir as mybir
    from concourse import bacc
    from concourse.tile import TileContext

    f32 = mybir.dt.float32
    bf = mybir.dt.bfloat16
    Alu = mybir.AluOpType
    Act = mybir.ActivationFunctionType

    nc = bacc.Bacc(
        "TRN2",
        target_bir_lowering=False,
        debug=False,
        enable_asserts=False,
        num_devices=N_CORES,
    )

    xcol_d = nc.dram_tensor("xcol", [2, 32, NPIX], bf, kind="ExternalInput")
    wcv_d = nc.dram_tensor("wcv", [128, NB * 64], bf, kind="ExternalInput")
    wde_d = nc.dram_tensor("wde", [128, NB * 256], bf, kind="ExternalInput")
    wdl_d = nc.dram_tensor("wdl", [128, NB * 152], bf, kind="ExternalInput")
    bv_d = nc.dram_tensor("bvec", [64, NB * 5], f32, kind="ExternalInput")
    yout_d = nc.dram_tensor("yout", [NB, 8, NPIX], bf, kind="ExternalOutput")

    with TileContext(nc) as tc:
        with (
            tc.tile_pool(name="const", bufs=1) as cpool,
            tc.tile_pool(name="psum", bufs=3, space="PSUM") as ppool,
        ):
            XC = cpool.tile([128, NPIX], bf, name="XC")
            WCV = cpool.tile([128, NB * 64], bf, name="WCV")
            WDE = cpool.tile([128, NB * 256], bf, name="WDE")
            WDL = cpool.tile([128, NB * 152], bf, name="WDL")
            BV = cpool.tile([64, NB * 5], f32, name="BV")

            # taps rows: ch0 at 0..17 (+pad to 31), ch1 at 64..81 (+pad),
            # zero rows shipped pre-padded from the host (no memset on the
            # critical path). XC is DMA'd in chunk-pair stages so the first
            # conv only waits for cols 0:1024 (subtile deps).
            nc.sync.dma_start(out=WCV[:, :], in_=wcv_d.ap())
            _sl = slice(0, CPW)
            nc.sync.dma_start(out=XC[0:32, _sl], in_=xcol_d.ap()[0][:, _sl])
            nc.sync.dma_start(out=XC[64:96, _sl], in_=xcol_d.ap()[1][:, _sl])
            nc.sync.dma_start(out=BV[:, :], in_=bv_d.ap())
            nc.sync.dma_start(out=WDE[:, :], in_=wde_d.ap())
            nc.sync.dma_start(out=WDL[:, :], in_=wdl_d.ap())
            for _cp in range(1, NCP):
                _sl = slice(_cp * CPW, (_cp + 1) * CPW)
                nc.sync.dma_start(out=XC[0:32, _sl], in_=xcol_d.ap()[0][:, _sl])
                nc.sync.dma_start(out=XC[64:96, _sl], in_=xcol_d.ap()[1][:, _sl])

            # Fixed per-lane activation tiles (reused across units; WAR deps
            # tracked by the tile framework). Free dim holds the chunk-pair:
            # [:, 0:512] = even chunk, [:, 512:1024] = odd chunk.
            lanes = []
            for i in range(LANES):
                h01 = cpool.tile([128, CPW], bf, name=f"h01L{i}")
                h23 = cpool.tile([128, CPW], bf, name=f"h23L{i}")
                h4y = cpool.tile([128, CPW], bf, name=f"h4yL{i}")
                # s5 reads h23[64:128] before r3 writes it (stale previous
                # tenant, nullified by zero weight rows) - make the first
                # tenant finite.
                nc.gpsimd.memset(h23[64:128, :], 0.0)
                lanes.append((h01, h23, h4y))

            ecnt = [0]

            def act_engine():
                c = ecnt[0]
                ecnt[0] += 1
                return (c % 11) % 2 == 0  # 6 of every 11 ops on ACT

            def relu1(out_ap, in_ap, bias_ap):
                if act_engine():
                    nc.scalar.activation(out_ap, in_ap, Act.Relu,
                                         bias=bias_ap, scale=1.0)
                else:
                    nc.vector.tensor_scalar(
                        out=out_ap, in0=in_ap, scalar1=bias_ap, scalar2=0.0,
                        op0=Alu.add, op1=Alu.max)

            def relu_op(out_ap, in_ap, bias_ap, split=False):
                if split:
                    # tail units: halve the latency per chain level so the
                    # pipeline drain is shallower
                    relu1(out_ap[:, 0:CW], in_ap[:, 0:CW], bias_ap)
                    relu1(out_ap[:, CW:CPW], in_ap[:, CW:CPW], bias_ap)
                else:
                    relu1(out_ap, in_ap, bias_ap)

            def copy1(out_ap, in_ap):
                if act_engine():
                    nc.scalar.copy(out_ap, in_ap)
                else:
                    nc.vector.tensor_copy(out_ap, in_ap)

            def copy_op(out_ap, in_ap, split=False):
                if split:
                    copy1(out_ap[:, 0:CW], in_ap[:, 0:CW])
                    copy1(out_ap[:, CW:CPW], in_ap[:, CW:CPW])
                else:
                    copy1(out_ap, in_ap)

            units = [(br, cp) for cp in range(NCP) for br in range(NB)]
            NU = len(units)
            state = [dict() for _ in range(NU)]
            EH, OH = slice(0, CW), slice(CW, CPW)

            def wde(br, c0, c1):
                return WDE[:, br * 256 + c0: br * 256 + c1]

            def wdl(br, c0, c1):
                return WDL[:, br * 152 + c0: br * 152 + c1]

            def phase(u, ph):
                br, cp = units[u]
                tail = u >= NU - 3
                st = state[u]
                h01, h23, h4y = lanes[u % LANES]
                px = [slice(cp * CPW + k * CW, cp * CPW + (k + 1) * CW)
                      for k in range(2)]
                halves = (EH, OH)

                def bias(l):
                    j = br * 5 + l
                    return BV[:, j:j + 1]

                if ph == 0:
                    o1 = ppool.tile([128, CPW], f32, name=f"o1_{u}", tag="x")
                    st["o1"] = o1
                    rb = slice(0, 32) if br < MODES else slice(64, 96)
                    lhs = WCV[rb, br * 64:(br + 1) * 64]
                    for k in range(2):
                        nc.tensor.matmul(o1[0:64, halves[k]], lhsT=lhs,
                                         rhs=XC[rb, px[k]],
                                         start=True, stop=True)
                    relu_op(h01[0:64, :], o1[0:64, :], bias(0), split=tail)
                elif ph == 1:
                    a1 = ppool.tile([128, CPW], f32, name=f"a1_{u}", tag="x")
                    st["a1"] = a1
                    lhs = wde(br, 0, 64)[0:64, :]
                    for k in range(2):
                        nc.tensor.matmul(a1[0:64, halves[k]], lhsT=lhs,
                                         rhs=h01[0:64, halves[k]],
                                         start=True, stop=True)
                    relu_op(h01[64:128, :], a1[0:64, :], bias(1), split=tail)
                elif ph == 2:
                    o2 = ppool.tile([128, CPW], f32, name=f"o2_{u}", tag="x")
                    a2 = ppool.tile([128, CPW], f32, name=f"a2_{u}", tag="x")
                    st["o2"], st["a2"] = o2, a2
                    for k in range(2):
                        nc.tensor.matmul(o2[0:64, halves[k]],
                                         lhsT=wde(br, 64, 128),
                                         rhs=h01[:, halves[k]],
                                         start=True, stop=True)
                    for k in range(2):
                        nc.tensor.matmul(a2[0:64, halves[k]],
                                         lhsT=wde(br, 128, 192),
                                         rhs=h01[:, halves[k]],
                                         start=True, stop=False)
                    relu_op(h23[0:64, :], o2[0:64, :], bias(2), split=tail)
                elif ph == 3:
                    a2 = st["a2"]
                    for k in range(2):
                        nc.tensor.matmul(a2[0:64, halves[k]],
                                         lhsT=wde(br, 192, 256),
                                         rhs=h23[:, halves[k]],
                                         start=False, stop=True)
                    relu_op(h23[64:128, :], a2[0:64, :], bias(3), split=tail)
                elif ph == 4:
                    dd = ppool.tile([128, CPW], f32, name=f"dd_{u}", tag="y",
                                    bufs=1)
                    st["dd"] = dd
                    for k in range(2):
                        nc.tensor.matmul(dd[0:72, halves[k]],
                                         lhsT=wdl(br, 0, 72),
                                         rhs=h01[:, halves[k]],
                                         start=True, stop=False)
                    for k in range(2):
                        nc.tensor.matmul(dd[0:72, halves[k]],
                                         lhsT=wdl(br, 72, 144),
                                         rhs=h23[:, halves[k]],
                                         start=False, stop=True)
                    relu_op(h4y[0:64, :], dd[0:64, :], bias(4), split=tail)
                else:
                    dd = st["dd"]
                    for k in range(2):
                        nc.tensor.matmul(dd[64:72, halves[k]],
                                         lhsT=wdl(br, 144, 152)[0:64, :],
                                         rhs=h4y[0:64, halves[k]],
                                         start=False, stop=True,
                                         skip_group_check=True)
                    copy_op(h4y[64:72, :], dd[64:72, :], split=tail)
                    nc.sync.dma_start(
                        out=yout_d.ap()[br, :, cp * CPW:(cp + 1) * CPW],
                        in_=h4y[64:72, :])

            NPH = 6
            for step in range(NU + NPH - 1):
                for ph in reversed(range(NPH)):
                    u = step - ph
                    if 0 <= u < NU:
                        phase(u, ph)

    nc.compile()
    return nc


def _get_bass():
    if "nc" not in _BASS_CACHE:
        _BASS_CACHE["nc"] = _build_bass()
    return _BASS_CACHE["nc"]


def _im2col_core(x, prev_x, r, b):
    """Per-core input: [2 (channel c), 18 (ci*9+dy*3+dx), 4096] bf16."""
    cols = np.zeros((2, 32, NPIX), np.float32)
    for c in range(2):
        xc = np.stack([x[b, c], prev_x[b, c]])  # [2, 64, 64]
        rot = np.rot90(xc, k=r, axes=(1, 2))
        padd = np.pad(rot, ((0, 0), (0, PAD), (0, PAD)), mode="edge")
        for ci in range(2):
            for dy in range(3):
                for dx in range(3):
                    cols[c, ci * 9 + dy * 3 + dx] = padd[
                        ci, dy: dy + H, dx: dx + H
                    ].reshape(-1)
    return cols.astype(BF16)


def _prep_weights(W1, B1, W2, B2, W3, B3, W4, B4, W5, B5, W6, B6):
    wcv = np.zeros((128, NB * 64), np.float32)
    for br in range(NB):
        r0 = 0 if br < MODES else 64
        # W1[br]: [64(out), 2(ci), 3, 3] -> rows ci*9+dy*3+dx, cols out
        wcv[r0:r0 + 18, br * 64:(br + 1) * 64] = (
            W1[br].transpose(1, 2, 3, 0).reshape(18, 64))

    wde = np.zeros((128, NB * 256), np.float32)
    wdl = np.zeros((128, NB * 152), np.float32)
    for br in range(NB):
        be, bl = br * 256, br * 152
        w2t = W2[br].T           # [64, 64]
        w3t = W3[br].T           # [128, 64]
        w4t = W4[br].T           # [192, 64]
        w5t = W5[br].T           # [256, 64]
        w6t = W6[br].T           # [320, 8]
        wde[0:64, be + 0: be + 64] = w2t
        wde[0:128, be + 64: be + 128] = w3t
        wde[0:128, be + 128: be + 192] = w4t[0:128]
        wde[0:64, be + 192: be + 256] = w4t[128:192]   # h3 rows stay 0
        wdl[0:128, bl + 0: bl + 64] = w5t[0:128]
        wdl[0:128, bl + 64: bl + 72] = w6t[0:128]
        wdl[0:128, bl + 72: bl + 136] = w5t[128:256]
        wdl[0:128, bl + 136: bl + 144] = w6t[128:256]
        wdl[0:64, bl + 144: bl + 152] = w6t[256:320]

        bvec = np.zeros((64, NB * 5), np.float32)
    for br in range(NB):
        for j, bb in enumerate((B1, B2, B3, B4, B5)):
            bvec[:, br * 5 + j] = bb[br]
    return wcv.astype(BF16), wde.astype(BF16), wdl.astype(BF16), bvec


def _postprocess(y_per_core, B6):
    """y_per_core[core] = yout [6, 8, 4096] bf16 (pre-B6); core = r*2 + b."""
    out = np.zeros((B, OUT_C, SCALE * H, SCALE * H), np.float32)
    for core in range(N_CORES):
        r, b = core // B, core % B
        y6 = np.asarray(y_per_core[core]).astype(np.float32) + B6[:, :, None]
        y6 = y6.reshape(NB, 8, H, H)
        z = np.round(np.tanh(y6) * np.float32(127.0))
        zz = (
            z.reshape(NB, OUT_C, SCALE, SCALE, H, H)
            .transpose(0, 1, 4, 2, 5, 3)
            .reshape(NB, OUT_C, SCALE * H, SCALE * H)
        )
        un = np.rot90(zz, k=(4 - r) % 4, axes=(2, 3))
        out[b] += un.sum(axis=0, dtype=np.float32)
    out /= np.float32(IN_C)
    return out


def kernel(x, prev_x, W1, B1, W2, B2, W3, B3, W4, B4, W5, B5, W6, B6,
           _trace=False):
    from concourse.bass_utils import run_bass_kernel_spmd

    args = [np.ascontiguousarray(np.asarray(a), dtype=np.float32) for a in
            (x, prev_x, W1, B1, W2, B2, W3, B3, W4, B4, W5, B5, W6, B6)]
    x, prev_x, W1, B1, W2, B2, W3, B3, W4, B4, W5, B5, W6, B6 = args

    wcv, wde, wdl, bvec = _prep_weights(W1, B1, W2, B2, W3, B3, W4, B4, W5, B5,
                                  W6, B6)

    in_maps = []
    for core in range(N_CORES):
        r, b = core // B, core % B
        in_maps.append(
            {
                "xcol": _im2col_core(x, prev_x, r, b),
                "wcv": wcv,
                "wde": wde,
                "wdl": wdl,
                "bvec": bvec,
            }
        )

    nc = _get_bass()
    if _trace:
        # Warmup execution: the device DVFS state alternates between runs;
        # a throwaway run first makes the traced run's clock state
        # reproducible.
        run_bass_kernel_spmd(nc, in_maps, core_ids=list(range(N_CORES)),
                             trace=False)
    res = run_bass_kernel_spmd(
        nc, in_maps, core_ids=list(range(N_CORES)), trace=_trace
    )
    _BASS_CACHE["last_results"] = res
    return _postprocess([res.results[c]["yout"] for c in range(N_CORES)], B6)

